# revision 1
# baseline (speedup 1.0000x reference)
"""Trainium2 Bass kernel for nn_LRSVConv (low-rank spatially-varying conv).

Computes, for full inputs
    x            [8, 32, 256, 256]  f32
    conv_w       [192, 32, 3, 3]    f32   (192 = RANK(3) * C_OUT(64))
    kernel_weight[2, 256, 256]      f32
the reference:
    y   = conv2d(x, conv_w, stride 1, pad 1)      # [8, 192, 256, 256]
    y   = y.reshape(8, 3, 64, 256, 256)
    out = y[:,0] + kw[0]*y[:,1] + kw[1]*y[:,2]    # [8, 64, 256, 256]

Strategy: spatial (H) sharding across 8 cores - each core computes a band of
32 output rows for ALL batches, so the per-pixel blend weights (which are
batch-independent) are loaded/broadcast once per core and reused 8x.

Per core:
  - imcol tile [96, 32*258]: 3 kh-shifted replicas of the padded input rows
    (partition dim = (kh, c_in)), padded W=258 so kw shifts are free-dim
    offsets and no edge handling is needed.
  - conv: per supertile (4 output rows = 1024 px, split into 2 blocks of
    512 px), per rank r and kw: one K=96, M=64, N=512 fp32 matmul per block,
    the two blocks on opposite column halves of the PE array (concurrent via
    col tiling), accumulating in PSUM banks A/B/C (one per rank); psum rows
    = (block, c_out).
  - blend: t1 = B * sv1_bcast, t2 = C * sv2_bcast on DVE; t1 accumulated
    onto A via an identity matmul on the (otherwise busier) TensorE;
    out = A + t2 on DVE (fused PSUM evacuation).
  - sv broadcast tiles are prepared host-side ([128, 4096] per rank: rows
    (block, c) x band pixels) - tiny input, avoids on-device partition
    broadcast which no engine does well.
"""

import os

import numpy as np

B, C_IN, C_OUT, RANK, IMG = 8, 32, 64, 3, 256
N_CORES = 8
BAND = IMG // N_CORES          # 32 output rows per core
WP = IMG + 2                   # padded width 258
ROWS_IN = BAND + 2             # input rows needed per band (with halo)
SUPER = 8                      # supertiles per (batch, band): 4 rows each
SROWS = BAND // SUPER          # 4 image rows per supertile
NBLK = 512                     # pixels per matmul block (2 image rows)

_F32 = np.float32

# "pe": rank-1 partial added into PSUM A by an identity matmul on TensorE
# "dve": both adds on VectorE (simpler, more DVE load)
BLEND_MODE = os.environ.get("KERNEL_BLEND", "pe")
NB = int(os.environ.get("KERNEL_NB", str(B)))  # batches to process (debug knob)


def _build_bass():
    import concourse.mybir as mybir
    import concourse.tile as tile
    from concourse import bacc

    f32 = mybir.dt.float32
    # float32r: single-pass PE fp32 (1 cyc/row at N>=256 vs 4 for fp32)
    f32r = mybir.dt.float32r
    nc = bacc.Bacc("TRN2", target_bir_lowering=False, debug=False)

    xs_t = nc.dram_tensor("xs", (B, C_IN, ROWS_IN * WP), f32r, kind="ExternalInput")
    # wtBC[kw]: [96, (rank1|rank2)]; wtA[kw, q]: [96, (w0|0) or (0|w0)]
    wtbc_t = nc.dram_tensor("wtbc", (3, 96, 128), f32r, kind="ExternalInput")
    wta_t = nc.dram_tensor("wta", (3, 2, 96, 128), f32r, kind="ExternalInput")
    # S12: rows 0:64 = sv1, rows 64:128 = sv2; cols = (supertile, block, j)
    svb_t = nc.dram_tensor("svb", (128, SUPER * 2 * NBLK), f32, kind="ExternalInput")
    # identII[q]: cols 64q:64q+64 hold [I64; I64] (sum the two 64-row halves)
    id_t = nc.dram_tensor("ident", (2, 128, 128), f32r, kind="ExternalInput")
    out_t = nc.dram_tensor("out", (B, C_OUT, BAND, IMG), f32, kind="ExternalOutput")

    xs = xs_t.ap()
    out_r = out_t.ap().rearrange(
        "b c (t q r) w -> b q c t (r w)", t=SUPER, q=2, r=SROWS // 2
    )

    with tile.TileContext(nc) as tc:
        with (
            tc.tile_pool(name="const", bufs=1) as cpool,
            tc.tile_pool(name="imcol", bufs=2) as ipool,
            tc.tile_pool(name="psum", bufs=2, space="PSUM") as ppool,
            tc.tile_pool(name="tmp", bufs=3) as tpool,
            tc.tile_pool(name="outp", bufs=4) as opool,
        ):
            wtbc_sb = cpool.tile([96, 3, 128], f32r)
            nc.sync.dma_start(wtbc_sb[:], wtbc_t.ap().rearrange("k p m -> p k m"))
            wta_sb = cpool.tile([96, 3, 2, 128], f32r)
            nc.sync.dma_start(wta_sb[:], wta_t.ap().rearrange("k q p m -> p k q m"))
            svb_sb = cpool.tile([128, SUPER * 2 * NBLK], f32)
            nc.sync.dma_start(svb_sb[:], svb_t.ap())
            id_sb = cpool.tile([128, 2, 128], f32r)
            nc.sync.dma_start(id_sb[:], id_t.ap().rearrange("q p m -> p q m"))

            for b in range(NB):
                imcol = ipool.tile([96, BAND * WP], f32r, tag="imcol")
                for kh in range(3):
                    nc.sync.dma_start(
                        imcol[32 * kh : 32 * kh + 32, :],
                        xs[b, :, kh * WP : kh * WP + BAND * WP],
                    )
                imv = imcol.rearrange("p (h w) -> p h w", w=WP)

                for t in range(SUPER):
                    bc = ppool.tile([128, 2 * NBLK], f32, tag="bc")
                    a2 = ppool.tile([128, NBLK], f32, tag="a2")
                    for kw in range(3):
                        for q in range(2):
                            hl = SROWS * t + 2 * q
                            rhs = imv[:, hl : hl + 2, kw : kw + IMG]
                            nc.tensor.matmul(
                                bc[:, NBLK * q : NBLK * (q + 1)],
                                wtbc_sb[:, kw, :],
                                rhs,
                                start=(kw == 0),
                                stop=(kw == 2),
                            )
                            nc.tensor.matmul(
                                a2[:],
                                wta_sb[:, kw, q, :],
                                rhs,
                                start=(kw == 0 and q == 0),
                                stop=False,
                            )

                    # m = [sv1*y1 ; sv2*y2] for both blocks, one 128-row op
                    m = tpool.tile([128, 2 * NBLK], f32r, tag="m")
                    nc.vector.tensor_tensor(
                        m[:],
                        bc,
                        svb_sb[:, 2 * NBLK * t : 2 * NBLK * (t + 1)],
                        mybir.AluOpType.mult,
                    )
                    # fold the two 64-row halves of m into a2 rows (q*64..)
                    for q in range(2):
                        nc.tensor.matmul(
                            a2[:],
                            id_sb[:, q, :],
                            m[:, NBLK * q : NBLK * (q + 1)],
                            start=False,
                            stop=(q == 1),
                        )
                    out_sb = opool.tile([128, NBLK], f32, tag="out_sb")
                    nc.scalar.copy(out_sb[:], a2[:])
                    for q in range(2):
                        nc.sync.dma_start(
                            out_r[b, q, :, t, :], out_sb[64 * q : 64 * q + 64, :]
                        )
    nc.compile()
    return nc


_CACHE = {}


def _get_bass():
    if "nc" not in _CACHE:
        _CACHE["nc"] = _build_bass()
    return _CACHE["nc"]


def _prep_shards(x, conv_w, kernel_weight):
    x = np.asarray(x, dtype=_F32)
    conv_w = np.asarray(conv_w, dtype=_F32)
    kernel_weight = np.asarray(kernel_weight, dtype=_F32)

    x_pad = np.pad(x, ((0, 0), (0, 0), (1, 1), (1, 1)))
    # w[kh, c, kw, (r, m)] from conv_w[(r m), c, kh, kw]
    wt = conv_w.transpose(2, 1, 3, 0).reshape(96, 3, RANK * C_OUT)
    wtbc = np.ascontiguousarray(
        wt[:, :, C_OUT:].reshape(96, 3, 128).transpose(1, 0, 2)
    )  # [kw, 96, (r1|r2)]
    wta = np.zeros((3, 2, 96, 128), dtype=_F32)
    for q in range(2):
        wta[:, q, :, 64 * q : 64 * q + 64] = wt[:, :, :C_OUT].transpose(1, 0, 2)
    ident = np.zeros((2, 128, 128), dtype=_F32)
    for q in range(2):
        ident[q, 0:64, 64 * q : 64 * q + 64] = np.eye(64, dtype=_F32)
        ident[q, 64:128, 64 * q : 64 * q + 64] = np.eye(64, dtype=_F32)

    in_maps = []
    for i in range(N_CORES):
        h0 = BAND * i
        shard = np.ascontiguousarray(
            x_pad[:, :, h0 : h0 + ROWS_IN, :]
        ).reshape(B, C_IN, ROWS_IN * WP)
        band = kernel_weight[:, h0 : h0 + BAND, :]          # [2, 32, 256]
        # svb[64r+c, (t, q, j)] = band[r, row(t, q, j)]
        arr = band.reshape(2, SUPER, 2 * NBLK)              # [r, t, (q j)]
        svb = np.broadcast_to(
            arr[:, None, :, :], (2, C_OUT, SUPER, 2 * NBLK)
        ).reshape(128, SUPER * 2 * NBLK)
        svb = np.ascontiguousarray(svb)
        in_maps.append(
            {"xs": shard, "wtbc": wtbc, "wta": wta, "svb": svb, "ident": ident}
        )
    return in_maps


def run(inputs, trace=False):
    """Run the sharded bass kernel; returns (out_full, BassKernelResults)."""
    from concourse.bass_utils import run_bass_kernel_spmd

    in_maps = _prep_shards(**inputs)
    nc = _get_bass()
    res = run_bass_kernel_spmd(
        nc, in_maps, core_ids=list(range(N_CORES)), trace=trace
    )
    out = np.empty((B, C_OUT, IMG, IMG), dtype=_F32)
    for i in range(N_CORES):
        out[:, :, BAND * i : BAND * (i + 1), :] = res.results[i]["out"]
    return out, res


def kernel(x, conv_w, kernel_weight):
    out, _ = run({"x": x, "conv_w": conv_w, "kernel_weight": kernel_weight})
    return out



# revision 4
# speedup vs baseline: 1.5752x; 1.5752x over previous
"""Trainium2 Bass kernel for nn_LRSVConv (low-rank spatially-varying conv).

Computes, for full inputs
    x            [8, 32, 256, 256]  f32
    conv_w       [192, 32, 3, 3]    f32   (192 = RANK(3) * C_OUT(64))
    kernel_weight[2, 256, 256]      f32
the reference:
    y   = conv2d(x, conv_w, stride 1, pad 1)      # [8, 192, 256, 256]
    y   = y.reshape(8, 3, 64, 256, 256)
    out = y[:,0] + kw[0]*y[:,1] + kw[1]*y[:,2]    # [8, 64, 256, 256]

Strategy: spatial (H) sharding across 8 cores - each core computes a band of
32 output rows for ALL batches, so the per-pixel blend weights (which are
batch-independent) are loaded once per core and reused 8x.

v3 design (vs the v1 full-array f32r kernel):
  - bf16 inputs/weights (host-converted; f32 PSUM accumulation).
  - PE column tiling (128x64 mode): every matmul has M=64 and targets one
    PSUM partition half; the two column tiles stream CONCURRENTLY (measured:
    a [96,64,512] pair completes in the time of one [96,128,512]).
  - Per supertile t (4 image rows = 2 blocks q of 512 px):
    18 conv matmuls: for kw, q, rank: [K=96, M=64, N=512] with out partition
    half q, accumulating rank r1 -> AB[:, 0:512], r2 -> AB[:, 512:1024],
    r0 -> C. After them, PSUM is rank-aligned: partition (64q+c).
  - blend: DVE m = AB * svAB (per-pixel sv weights broadcast host-side),
    written bf16; folded into C by 4 column-tiled identity matmuls
    (C[half] += m1, m2). No cross-partition traffic anywhere.
  - software-pipelined PE stream: identity folds of supertile t-1 are
    emitted AFTER the conv matmuls of supertile t, so the PE never waits
    on the DVE multiply.
  - input band for batch b+1 prefetched before batch b's output DMAs
    enter the queues (removes the ~10.5us per-batch PE stall of v1).
"""

import os

import numpy as np
from ml_dtypes import bfloat16 as np_bf16

B, C_IN, C_OUT, RANK, IMG = 8, 32, 64, 3, 256
N_CORES = 8
BAND = IMG // N_CORES          # 32 output rows per core
WP = IMG + 2                   # padded width 258
ROWS_IN = BAND + 2             # input rows needed per band (with halo)
SUPER = 8                      # supertiles per (batch, band): 4 rows each
SROWS = BAND // SUPER          # 4 image rows per supertile
NBLK = 512                     # pixels per matmul block (2 image rows)

_F32 = np.float32

NB = int(os.environ.get("KERNEL_NB", str(B)))  # batches to process (debug knob)


def _build_bass():
    import concourse.mybir as mybir
    import concourse.tile as tile
    from concourse import bacc

    f32 = mybir.dt.float32
    bf16 = mybir.dt.bfloat16
    nc = bacc.Bacc("TRN2", target_bir_lowering=False, debug=False)

    xs_t = nc.dram_tensor("xs", (B, C_IN, ROWS_IN * WP), bf16, kind="ExternalInput")
    # wc[(kh,cin), (r,kw), c]: 9 column-tile stationaries of 64 channels
    wc_t = nc.dram_tensor("wc", (96, 9, 64), bf16, kind="ExternalInput")
    # idq[h]: [128, 64] identity slabs: id0 = [I64; 0], id1 = [0; I64]
    id_t = nc.dram_tensor("ident", (2, 128, 64), bf16, kind="ExternalInput")
    # svAB[(q,c), t, (s,j)]: per-pixel blend weights for ranks 1 (s=0), 2 (s=1)
    svb_t = nc.dram_tensor("svb", (128, SUPER, 2 * NBLK), f32, kind="ExternalInput")
    out_t = nc.dram_tensor("out", (B, C_OUT, BAND, IMG), f32, kind="ExternalOutput")

    xs = xs_t.ap()
    out_r = out_t.ap().rearrange(
        "b c (t q r) w -> b q c t (r w)", t=SUPER, q=2, r=SROWS // 2
    )

    with tile.TileContext(nc) as tc:
        with (
            tc.tile_pool(name="const", bufs=1) as cpool,
            tc.tile_pool(name="imcol", bufs=2) as ipool,
            tc.tile_pool(name="psum", bufs=2, space="PSUM") as ppool,
            tc.tile_pool(name="tmp", bufs=3) as tpool,
            tc.tile_pool(name="outp", bufs=4) as opool,
        ):
            wc_sb = cpool.tile([96, 9, 64], bf16)
            nc.sync.dma_start(wc_sb[:], wc_t.ap())
            id_sb = cpool.tile([128, 2, 64], bf16)
            nc.sync.dma_start(id_sb[:], id_t.ap().rearrange("h p m -> p h m"))
            svb_sb = cpool.tile([128, SUPER, 2 * NBLK], f32)
            nc.sync.dma_start(svb_sb[:], svb_t.ap())

            def load_imcol(b):
                t = ipool.tile([96, BAND * WP], bf16, tag="imcol")
                for kh in range(3):
                    nc.sync.dma_start(
                        t[32 * kh : 32 * kh + 32, :],
                        xs[b, :, kh * WP : kh * WP + BAND * WP],
                    )
                return t

            def emit_conv(imv, t):
                """18 column-tiled conv matmuls for supertile t; returns (AB, C)."""
                ab = ppool.tile([128, 2 * NBLK], f32, tag="ab")
                c = ppool.tile([128, NBLK], f32, tag="c")
                hl = SROWS * t
                for kw in range(3):
                    st, sp = kw == 0, kw == 2
                    for q in range(2):
                        rhs = imv[:, hl + 2 * q : hl + 2 * q + 2, kw : kw + IMG]
                        o = 64 * q
                        nc.tensor.matmul(
                            ab[o : o + 64, 0:NBLK],
                            wc_sb[:, 3 * 1 + kw, :], rhs, start=st, stop=sp,
                        )
                        nc.tensor.matmul(
                            ab[o : o + 64, NBLK : 2 * NBLK],
                            wc_sb[:, 3 * 2 + kw, :], rhs, start=st, stop=sp,
                        )
                        nc.tensor.matmul(
                            c[o : o + 64, :],
                            wc_sb[:, 3 * 0 + kw, :], rhs, start=st, stop=False,
                        )
                return ab, c

            def emit_blend_mult(ab, t):
                """DVE: m = AB * svAB  (psum f32 x sbuf f32 -> sbuf bf16)."""
                m = tpool.tile([128, 2 * NBLK], bf16, tag="m")
                nc.vector.tensor_tensor(
                    m[:], ab, svb_sb[:, t, :], mybir.AluOpType.mult
                )
                return m

            def emit_fold_out(c, m, b, t):
                """PE: C[half] += m1, m2 (4 col-tiled idents); ACT copy; DMA."""
                for s in range(2):
                    for q in range(2):
                        o = 64 * q
                        nc.tensor.matmul(
                            c[o : o + 64, :],
                            id_sb[:, q, :],
                            m[:, NBLK * s : NBLK * (s + 1)],
                            start=False,
                            stop=(s == 1),
                        )
                out_sb = opool.tile([128, NBLK], f32, tag="out_sb")
                nc.scalar.copy(out_sb[:], c[:])
                for q in range(2):
                    nc.sync.dma_start(
                        out_r[b, q, :, t, :], out_sb[64 * q : 64 * q + 64, :]
                    )

            imcol = load_imcol(0)
            pend = None  # (C, m, b, t) of the previous supertile
            for b in range(NB):
                imcol_nxt = load_imcol(b + 1) if b + 1 < NB else None
                imv = imcol.rearrange("p (h w) -> p h w", w=WP)
                for t in range(SUPER):
                    ab, c = emit_conv(imv, t)
                    m = emit_blend_mult(ab, t)
                    if pend is not None:
                        emit_fold_out(*pend)
                    pend = (c, m, b, t)
                imcol = imcol_nxt
            emit_fold_out(*pend)
    nc.compile()
    return nc


_CACHE = {}


def _get_bass():
    if "nc" not in _CACHE:
        _CACHE["nc"] = _build_bass()
    return _CACHE["nc"]


def _prep_shards(x, conv_w, kernel_weight):
    x = np.asarray(x, dtype=_F32)
    conv_w = np.asarray(conv_w, dtype=_F32)
    kernel_weight = np.asarray(kernel_weight, dtype=_F32)

    x_pad = np.pad(x, ((0, 0), (0, 0), (1, 1), (1, 1))).astype(np_bf16)
    # wc[(kh,cin), (r,kw), c] from conv_w[(r c), cin, kh, kw]
    wc = np.ascontiguousarray(
        conv_w.reshape(RANK, C_OUT, C_IN, 3, 3)
        .transpose(3, 2, 0, 4, 1)
        .reshape(96, 9, 64)
    ).astype(np_bf16)
    ident = np.zeros((2, 128, 64), dtype=np_bf16)
    ident[0, 0:64] = np.eye(64, dtype=np_bf16)
    ident[1, 64:128] = np.eye(64, dtype=np_bf16)

    in_maps = []
    for i in range(N_CORES):
        h0 = BAND * i
        shard = np.ascontiguousarray(
            x_pad[:, :, h0 : h0 + ROWS_IN, :]
        ).reshape(B, C_IN, ROWS_IN * WP)
        band = kernel_weight[:, h0 : h0 + BAND, :]          # [2, 32, 256]
        # svAB[64q+c, t, (s,j)] = band[s, 4t+2q+(j//256), j%256]
        tmp = band.reshape(2, SUPER, 2, NBLK)               # [s, t, q, j]
        svb = np.broadcast_to(
            tmp.transpose(2, 1, 0, 3)[:, None],             # [q, 1, t, s, j]
            (2, C_OUT, SUPER, 2, NBLK),
        ).reshape(128, SUPER, 2 * NBLK)
        svb = np.ascontiguousarray(svb)
        in_maps.append({"xs": shard, "wc": wc, "ident": ident, "svb": svb})
    return in_maps


def run(inputs, trace=False):
    """Run the sharded bass kernel; returns (out_full, BassKernelResults)."""
    from concourse.bass_utils import run_bass_kernel_spmd

    in_maps = _prep_shards(**inputs)
    nc = _get_bass()
    res = run_bass_kernel_spmd(
        nc, in_maps, core_ids=list(range(N_CORES)), trace=trace
    )
    out = np.empty((B, C_OUT, IMG, IMG), dtype=_F32)
    for i in range(N_CORES):
        out[:, :, BAND * i : BAND * (i + 1), :] = res.results[i]["out"]
    return out, res


def kernel(x, conv_w, kernel_weight):
    out, _ = run({"x": x, "conv_w": conv_w, "kernel_weight": kernel_weight})
    return out


# revision 5
# speedup vs baseline: 2.0688x; 1.3133x over previous
"""Trainium2 Bass kernel for nn_LRSVConv (low-rank spatially-varying conv).

Computes, for full inputs
    x            [8, 32, 256, 256]  f32
    conv_w       [192, 32, 3, 3]    f32   (192 = RANK(3) * C_OUT(64))
    kernel_weight[2, 256, 256]      f32
the reference:
    y   = conv2d(x, conv_w, stride 1, pad 1)      # [8, 192, 256, 256]
    y   = y.reshape(8, 3, 64, 256, 256)
    out = y[:,0] + kw[0]*y[:,1] + kw[1]*y[:,2]    # [8, 64, 256, 256]

Strategy: spatial (H) sharding across 8 cores - each core computes a band of
32 output rows for ALL batches, so the per-pixel blend weights (which are
batch-independent) are loaded once per core and reused 8x.

v4 design:
  - bf16 inputs/weights (host-converted; f32 PSUM accumulation).
  - PE column tiling (128x64 mode): every matmul has M=64 and targets one
    PSUM partition half; the two column tiles stream CONCURRENTLY, so a
    [96,64,512] pair completes in the time of one [96,128,512]. The PE does
    ONLY the 18 conv matmuls per supertile (9 concurrent pair-slots).
  - Per supertile t (4 image rows = 2 blocks q of 512 px): accumulate
    rank r1 -> AB[:, 0:512], r2 -> AB[:, 512:1024], r0 -> C; PSUM ends up
    rank-aligned on partition (64q+c) with no cross-partition traffic.
  - blend fold spread across the otherwise-idle engines:
      DVE:    m = AB * svAB      [128,1024] psum*sbuf->sbuf
      DVE:    s = C + m1         [128,512]  psum+sbuf->sbuf
      GPSIMD: out_sb = s + m2    [128,512]  sbuf
  - software-pipelined: supertile t-1's fold ops are emitted after
    supertile t's conv matmuls, so no engine waits on another in
    steady state.
  - DMA order: weights + first input band BEFORE the sv tiles (which are
    split per supertile) so the first matmul starts ~8us in, not ~34us.
  - input band for batch b+1 prefetched before batch b's output DMAs
    enter the queues.
"""

import os

import numpy as np
from ml_dtypes import bfloat16 as np_bf16

B, C_IN, C_OUT, RANK, IMG = 8, 32, 64, 3, 256
N_CORES = 8
BAND = IMG // N_CORES          # 32 output rows per core
WP = IMG + 2                   # padded width 258
ROWS_IN = BAND + 2             # input rows needed per band (with halo)
SUPER = 8                      # supertiles per (batch, band): 4 rows each
SROWS = BAND // SUPER          # 4 image rows per supertile
NBLK = 512                     # pixels per matmul block (2 image rows)

_F32 = np.float32

NB = int(os.environ.get("KERNEL_NB", str(B)))  # batches to process (debug knob)


def _build_bass():
    import concourse.mybir as mybir
    import concourse.tile as tile
    from concourse import bacc

    f32 = mybir.dt.float32
    bf16 = mybir.dt.bfloat16
    nc = bacc.Bacc("TRN2", target_bir_lowering=False, debug=False)

    xs_t = nc.dram_tensor("xs", (B, C_IN, ROWS_IN * WP), bf16, kind="ExternalInput")
    # wc[(kh,cin), (r,kw), c]: 9 column-tile stationaries of 64 channels
    wc_t = nc.dram_tensor("wc", (96, 9, 64), bf16, kind="ExternalInput")
    # svAB[(q,c), t, (s,j)]: per-pixel blend weights for ranks 1 (s=0), 2 (s=1)
    svb_t = nc.dram_tensor("svb", (128, SUPER, 2 * NBLK), f32, kind="ExternalInput")
    out_t = nc.dram_tensor("out", (B, C_OUT, BAND, IMG), f32, kind="ExternalOutput")

    xs = xs_t.ap()
    out_r = out_t.ap().rearrange(
        "b c (t q r) w -> b q c t (r w)", t=SUPER, q=2, r=SROWS // 2
    )

    with tile.TileContext(nc) as tc:
        with (
            tc.tile_pool(name="const", bufs=1) as cpool,
            tc.tile_pool(name="imcol", bufs=2) as ipool,
            tc.tile_pool(name="psab", bufs=2, space="PSUM") as abpool,
            tc.tile_pool(name="psc", bufs=3, space="PSUM") as cpool_ps,
            tc.tile_pool(name="tmp", bufs=3) as tpool,
            tc.tile_pool(name="outp", bufs=4) as opool,
        ):
            wc_sb = cpool.tile([96, 9, 64], bf16)
            nc.sync.dma_start(wc_sb[:], wc_t.ap())

            def load_imcol(b):
                t = ipool.tile([96, BAND * WP], bf16, tag="imcol")
                for kh in range(3):
                    nc.sync.dma_start(
                        t[32 * kh : 32 * kh + 32, :],
                        xs[b, :, kh * WP : kh * WP + BAND * WP],
                    )
                return t

            imcol = load_imcol(0)

            # per-supertile sv tiles, loaded after the first input band so
            # they never gate the first matmuls
            svb_sbs = []
            for t in range(SUPER):
                sv = cpool.tile([128, 2 * NBLK], f32, tag=f"svb{t}")
                nc.sync.dma_start(sv[:], svb_t.ap()[:, t, :])
                svb_sbs.append(sv)

            def emit_conv(imv, t):
                """18 column-tiled conv matmuls for supertile t; returns (AB, C)."""
                ab = abpool.tile([128, 2 * NBLK], f32, tag="ab")
                c = cpool_ps.tile([128, NBLK], f32, tag="c")
                hl = SROWS * t
                for kw in range(3):
                    st, sp = kw == 0, kw == 2
                    for q in range(2):
                        rhs = imv[:, hl + 2 * q : hl + 2 * q + 2, kw : kw + IMG]
                        o = 64 * q
                        nc.tensor.matmul(
                            ab[o : o + 64, 0:NBLK],
                            wc_sb[:, 3 * 1 + kw, :], rhs, start=st, stop=sp,
                        )
                        nc.tensor.matmul(
                            ab[o : o + 64, NBLK : 2 * NBLK],
                            wc_sb[:, 3 * 2 + kw, :], rhs, start=st, stop=sp,
                        )
                        nc.tensor.matmul(
                            c[o : o + 64, :],
                            wc_sb[:, 3 * 0 + kw, :], rhs, start=st, stop=sp,
                        )
                return ab, c

            def emit_blend_mult(ab, t):
                """DVE: m = AB * svAB  (psum f32 x sbuf f32 -> sbuf f32)."""
                m = tpool.tile([128, 2 * NBLK], f32, tag="m")
                nc.vector.tensor_tensor(
                    m[:], ab, svb_sbs[t][:], mybir.AluOpType.mult
                )
                return m

            def emit_fold_out(c, m, b, t):
                """DVE: s = C + m1; GPSIMD: out = s + m2; DMA out."""
                s = tpool.tile([128, NBLK], f32, tag="s")
                nc.vector.tensor_tensor(s[:], c, m[:, 0:NBLK], mybir.AluOpType.add)
                out_sb = opool.tile([128, NBLK], f32, tag="out_sb")
                nc.gpsimd.tensor_tensor(
                    out_sb[:], s[:], m[:, NBLK : 2 * NBLK], mybir.AluOpType.add
                )
                for q in range(2):
                    nc.sync.dma_start(
                        out_r[b, q, :, t, :], out_sb[64 * q : 64 * q + 64, :]
                    )

            pend = None  # (C, m, b, t) of the previous supertile
            for b in range(NB):
                imcol_nxt = load_imcol(b + 1) if b + 1 < NB else None
                imv = imcol.rearrange("p (h w) -> p h w", w=WP)
                for t in range(SUPER):
                    ab, c = emit_conv(imv, t)
                    m = emit_blend_mult(ab, t)
                    if pend is not None:
                        emit_fold_out(*pend)
                    pend = (c, m, b, t)
                imcol = imcol_nxt
            emit_fold_out(*pend)
    nc.compile()
    return nc


_CACHE = {}


def _get_bass():
    if "nc" not in _CACHE:
        _CACHE["nc"] = _build_bass()
    return _CACHE["nc"]


def _prep_shards(x, conv_w, kernel_weight):
    x = np.asarray(x, dtype=_F32)
    conv_w = np.asarray(conv_w, dtype=_F32)
    kernel_weight = np.asarray(kernel_weight, dtype=_F32)

    x_pad = np.pad(x, ((0, 0), (0, 0), (1, 1), (1, 1))).astype(np_bf16)
    # wc[(kh,cin), (r,kw), c] from conv_w[(r c), cin, kh, kw]
    wc = np.ascontiguousarray(
        conv_w.reshape(RANK, C_OUT, C_IN, 3, 3)
        .transpose(3, 2, 0, 4, 1)
        .reshape(96, 9, 64)
    ).astype(np_bf16)

    in_maps = []
    for i in range(N_CORES):
        h0 = BAND * i
        shard = np.ascontiguousarray(
            x_pad[:, :, h0 : h0 + ROWS_IN, :]
        ).reshape(B, C_IN, ROWS_IN * WP)
        band = kernel_weight[:, h0 : h0 + BAND, :]          # [2, 32, 256]
        # svAB[64q+c, t, (s,j)] = band[s, 4t+2q+(j//256), j%256]
        tmp = band.reshape(2, SUPER, 2, NBLK)               # [s, t, q, j]
        svb = np.broadcast_to(
            tmp.transpose(2, 1, 0, 3)[:, None],             # [q, 1, t, s, j]
            (2, C_OUT, SUPER, 2, NBLK),
        ).reshape(128, SUPER, 2 * NBLK)
        svb = np.ascontiguousarray(svb)
        in_maps.append({"xs": shard, "wc": wc, "svb": svb})
    return in_maps


def run(inputs, trace=False):
    """Run the sharded bass kernel; returns (out_full, BassKernelResults)."""
    from concourse.bass_utils import run_bass_kernel_spmd

    in_maps = _prep_shards(**inputs)
    nc = _get_bass()
    res = run_bass_kernel_spmd(
        nc, in_maps, core_ids=list(range(N_CORES)), trace=trace
    )
    out = np.empty((B, C_OUT, IMG, IMG), dtype=_F32)
    for i in range(N_CORES):
        out[:, :, BAND * i : BAND * (i + 1), :] = res.results[i]["out"]
    return out, res


def kernel(x, conv_w, kernel_weight):
    out, _ = run({"x": x, "conv_w": conv_w, "kernel_weight": kernel_weight})
    return out
